# revision 17
# baseline (speedup 1.0000x reference)
"""Trainium2 Bass kernel for nn_BHS_SAGE (GNN message passing + dueling head).

v4: node-parallel K-split. Each NeuronCore owns a 128-node dst slice of ALL
128 graphs (instead of 16 whole graphs). The SAGE layers are node-local
(edges gathered on host into the per-core xe stream, exactly like the
data-parallel baseline), and the huge dueling-head GEMM contracts only this
core's K-slice — so the head weights shrink 8x per core (1.44 MB vs 19.9)
and the head matmuls run at full M=128. One 39 KB ReduceScatter sums the
head partials; each core then runs the tiny dueling tail for its 16 graphs.

Per-core pipeline (128 graphs x 128 dst nodes, F=32, H=128, DEG=16):
  A. z-edge = W_pool @ xe (xe = host-gathered edge-ordered x, bf16,
     [128=(q4,f32), (gb32, n128, d16)]), 512-col psum tiles
  B. DVE reduce_max over d=16 -> aggT [(q,f), (gb, n)], fused bias+relu
  D. ht = relu(W_self x + W_neigh agg + b_sage) -> [128h, (n, g)] bf16
  E. head: ps_adv[128g,12] (bf16 W) / ps_val[128g,64] (fp8-e4m3 W, x8192)
     accumulated over this core's 128 nodes x 128 h
  F. ReduceScatter(add) of [128, 76] -> [16, 76]; dueling tail -> out [16,12]
"""

import numpy as np

B, N, F, H, DEG = 128, 1024, 32, 128, 16
NCORES = 8
NS = N // NCORES          # 128 dst nodes per core
BL = B // NCORES          # 16 graphs per core (output/tail)
NA = 12                   # adv outputs (3 branches x 4 actions)
NV = 64                   # val hidden
NH = NA + NV
GB = B // 4               # 32 graph-blocks of 4 graphs on 128 partitions
VSCALE = 8192.0           # fp8 scale for W_v1

_CACHE = {}
LAST_RESULTS = None


def _build_program():
    import concourse.bacc as bacc
    import concourse.mybir as mybir
    import concourse.tile as tile

    f32 = mybir.dt.float32
    bf16 = mybir.dt.bfloat16
    fp8 = mybir.dt.float8e4
    Relu = mybir.ActivationFunctionType.Relu
    Alu = mybir.AluOpType

    nc = bacc.Bacc("TRN2", target_bir_lowering=False, debug=False,
                   num_devices=NCORES)

    ECOLS = GB * NS * DEG          # 65536 edge columns
    SLAB = 8192                    # edge cols per streamed xe tile
    NSLAB = ECOLS // SLAB          # 8

    # ---- kernel I/O ----
    xt_d = nc.declare_dram_parameter("xt", [128, GB * NS], bf16, isOutput=False)
    xe_d = nc.declare_dram_parameter("xe", [128, ECOLS], bf16, isOutput=False)
    wpool_d = nc.declare_dram_parameter("wpool_bd", [128, 128], bf16, isOutput=False)
    bpool_d = nc.declare_dram_parameter("bpool", [128, 1], f32, isOutput=False)
    wself_d = nc.declare_dram_parameter("wself_bd", [128, 4 * H], bf16, isOutput=False)
    wneigh_d = nc.declare_dram_parameter("wneigh_bd", [128, 4 * H], bf16, isOutput=False)
    bsage_d = nc.declare_dram_parameter("bsage", [128, 1], f32, isOutput=False)
    wha_d = nc.declare_dram_parameter("whead_adv", [128, NS * NA], bf16, isOutput=False)
    whv_d = nc.declare_dram_parameter("whead_val", [128, NS * NV], fp8, isOutput=False)
    heads_d = nc.declare_dram_parameter("heads", [128, NH], f32, isOutput=True)

    import os as _os
    _dbg = _os.environ.get("KDBG") == "1"
    if _dbg:
        dbg_aggT_d = nc.declare_dram_parameter("dbg_aggT", [128, GB * NS], bf16, isOutput=True)
        dbg_ht_d = nc.declare_dram_parameter("dbg_ht", [128, NS * B], bf16, isOutput=True)

    with tile.TileContext(nc) as tc:
        with (
            tc.tile_pool(name="const", bufs=1) as cpool,
            tc.tile_pool(name="big", bufs=1) as bigpool,
        ):
            # ---- constants / persistent tiles ----
            # (only wpool/bpool load up front; the rest queue behind the
            # first xe slab so stage A starts ASAP)
            wpool = cpool.tile([128, 128], bf16)
            nc.sync.dma_start(out=wpool[:], in_=wpool_d[:])
            bpool = cpool.tile([128, 1], f32)
            nc.sync.dma_start(out=bpool[:], in_=bpool_d[:])
            xt = cpool.tile([128, GB * NS], bf16)
            wself = cpool.tile([128, 4 * H], bf16)
            wneigh = cpool.tile([128, 4 * H], bf16)
            bsage = cpool.tile([128, 1], f32)
            wha = cpool.tile([128, NS * NA], bf16)
            whv = cpool.tile([128, NS * NV], fp8)

            ht = bigpool.tile([128, NS * B], bf16)        # [h, n*128+g] 4MB
            aggT = bigpool.tile([128, GB * NS], bf16)     # [(q,f), gb*128+n] 1MB

            # head psums allocated up-front so stage E overlaps stage D
            hd_ps_ctx = tc.tile_pool(name="hd_ps", bufs=1, space="PSUM")
            hd_ps = hd_ps_ctx.__enter__()
            ps_adv = hd_ps.tile([128, NA], f32)
            ps_val = hd_ps.tile([128, NV], f32)

            # ---- stage A+B: aggT = relu(max_d(W_pool @ x[src_d]) + b) ----
            # column order is (n, gb): aggT[:, n*GB+gb]
            with (
                tc.tile_pool(name="xe_sb", bufs=3) as xe_pool,
                tc.tile_pool(name="zb_sb", bufs=3) as zb_pool,
                tc.tile_pool(name="z_ps", bufs=2, space="PSUM") as z_ps,
            ):
                for s in range(NSLAB):
                    xe = xe_pool.tile([128, SLAB], bf16, tag="xe")
                    nc.sync.dma_start(
                        out=xe[:], in_=xe_d[:, s * SLAB:(s + 1) * SLAB])
                    if s == 0:
                        nc.sync.dma_start(out=xt[:], in_=xt_d[:])
                        nc.sync.dma_start(out=wself[:], in_=wself_d[:])
                        nc.sync.dma_start(out=wneigh[:], in_=wneigh_d[:])
                        nc.sync.dma_start(out=bsage[:], in_=bsage_d[:])
                        nc.sync.dma_start(out=wha[:], in_=wha_d[:])
                        nc.sync.dma_start(out=whv[:], in_=whv_d[:])
                    for blk in range(SLAB // 1024):   # 64 nodes x 16 d per blk
                        ps = z_ps.tile([128, 1024], f32, tag="zps")
                        for hf in range(2):           # moving-dim cap is 512
                            nc.tensor.matmul(
                                out=ps[:, hf * 512:(hf + 1) * 512],
                                lhsT=wpool[:],
                                rhs=xe[:, blk * 1024 + hf * 512:
                                        blk * 1024 + (hf + 1) * 512],
                                start=True, stop=True,
                            )
                        t = s * (SLAB // 1024) + blk
                        if t % 5 < 2:
                            # scalar-engine path: psum -> bf16, fast DVE reduce
                            zb = zb_pool.tile([128, 1024], bf16, tag="zb")
                            nc.scalar.copy(out=zb[:], in_=ps[:])
                            nc.vector.reduce_max(
                                out=aggT[:, t * 64:(t + 1) * 64],
                                in_=zb[:].rearrange("p (n d) -> p n d", d=DEG),
                                axis=mybir.AxisListType.X)
                        else:
                            nc.vector.reduce_max(
                                out=aggT[:, t * 64:(t + 1) * 64],
                                in_=ps[:].rearrange("p (n d) -> p n d", d=DEG),
                                axis=mybir.AxisListType.X)
            # fused bias + relu (per-partition bias); monotone max trick
            for h4 in range(4):
                nc.scalar.activation(
                    out=aggT[:, h4 * 1024:(h4 + 1) * 1024],
                    in_=aggT[:, h4 * 1024:(h4 + 1) * 1024],
                    func=Relu, bias=bpool[:])

            if _dbg:
                nc.sync.dma_start(out=dbg_aggT_d[:], in_=aggT[:])

            # ---- stage D: ht = relu(W_self x + W_neigh agg + b_sage) ----
            # n-major layout: ht[h, n*128 + q*32 + gb] -> stage E stationaries
            # are contiguous; D's writes land in 32-elem (64 B) runs
            htv = ht[:].rearrange("p (n qq gb) -> p qq n gb", qq=4, gb=GB)
            with tc.tile_pool(name="h_ps", bufs=2, space="PSUM") as h_ps:
                for q in range(4):
                    for ch in range(GB * NS // 512):   # 8 chunks of 512 cols
                        base = ch * 512
                        ps = h_ps.tile([128, 512], f32, tag="hps")
                        nc.tensor.matmul(
                            out=ps[:],
                            lhsT=wself[:, q * H:(q + 1) * H],
                            rhs=xt[:, base: base + 512],
                            start=True, stop=False)
                        nc.tensor.matmul(
                            out=ps[:],
                            lhsT=wneigh[:, q * H:(q + 1) * H],
                            rhs=aggT[:, base: base + 512],
                            start=False, stop=True)
                        nc.scalar.activation(
                            out=htv[:, q, 16 * ch:16 * ch + 16, :],
                            in_=ps[:].rearrange("p (n gb) -> p n gb", gb=GB),
                            func=Relu, bias=bsage[:])

            if _dbg:
                nc.sync.dma_start(out=dbg_ht_d[:], in_=ht[:])

            # ---- stage E: head psums += ht[:, n-col].T @ W ----
            # ht col block j*128..(j+1)*128 = node j, graphs (q*32+gb) order
            for j in range(NS):
                lhsT = ht[:, j * B:(j + 1) * B]
                nc.tensor.matmul(
                    out=ps_adv[:],
                    lhsT=lhsT,
                    rhs=wha[:, j * NA:(j + 1) * NA],
                    start=(j == 0), stop=(j == NS - 1),
                )
                nc.tensor.matmul(
                    out=ps_val[:],
                    lhsT=lhsT,
                    rhs=whv[:, j * NV:(j + 1) * NV],
                    start=(j == 0), stop=(j == NS - 1),
                )

            # ---- stage F: write partial heads (host sums across cores) ----
            with tc.tile_pool(name="tail", bufs=1) as tp:
                heads = tp.tile([128, NH], f32)
                nc.scalar.copy(out=heads[:, 0:NA], in_=ps_adv[:])
                nc.scalar.copy(out=heads[:, NA:NH], in_=ps_val[:])
                nc.sync.dma_start(out=heads_d[:], in_=heads[:])
            hd_ps_ctx.__exit__(None, None, None)
    nc.compile()
    return nc


def _make_in_maps(inputs):
    import ml_dtypes
    bf = ml_dtypes.bfloat16
    e4 = ml_dtypes.float8_e4m3

    x = np.asarray(inputs["x"], np.float32)
    src = np.asarray(inputs["src"], np.int32)
    W_pool = np.asarray(inputs["W_pool"], np.float32)
    b_pool = np.asarray(inputs["b_pool"], np.float32)
    W_self = np.asarray(inputs["W_self"], np.float32)
    W_neigh = np.asarray(inputs["W_neigh"], np.float32)
    b_sage = np.asarray(inputs["b_sage"], np.float32)
    W_adv = np.asarray(inputs["W_adv"], np.float32)
    b_adv = np.asarray(inputs["b_adv"], np.float32)
    W_v1 = np.asarray(inputs["W_v1"], np.float32)
    b_v1 = np.asarray(inputs["b_v1"], np.float32)
    W_v2 = np.asarray(inputs["W_v2"], np.float32)
    b_v2 = np.asarray(inputs["b_v2"], np.float32)
    W_v3 = np.asarray(inputs["W_v3"], np.float32)
    b_v3 = np.asarray(inputs["b_v3"], np.float32)

    # shared (replicated) tensors
    wpool_bd = np.kron(np.eye(4, dtype=np.float32), W_pool.T)                # [128, 128]
    wpool_bd = np.ascontiguousarray(wpool_bd).astype(bf)
    bpool = np.ascontiguousarray(np.tile(b_pool, 4)[:, None], np.float32)    # [128, 1]
    wself_bd = np.zeros((128, 4 * H), np.float32)                            # [128, 512]
    wneigh_bd = np.zeros((128, 4 * H), np.float32)
    for q in range(4):
        wself_bd[q * 32:(q + 1) * 32, q * H:(q + 1) * H] = W_self.T
        wneigh_bd[q * 32:(q + 1) * 32, q * H:(q + 1) * H] = W_neigh.T
    bsage = np.ascontiguousarray(b_sage[:, None])                            # [128, 1]
    shared = {
        "wpool_bd": wpool_bd, "bpool": bpool,
        "wself_bd": wself_bd.astype(bf), "wneigh_bd": wneigh_bd.astype(bf),
        "bsage": bsage,
    }

    # per-graph src indices (identical across graphs: dgl.batch of one graph)
    idxg = (src[: N * DEG] - 0).reshape(N, DEG)      # graph-0 global indices
    Wa = W_adv.reshape(NA, N, H)                     # [12, n, h]
    Wv = (W_v1 * VSCALE).reshape(NV, N, H)           # [64, n, h]

    in_maps = []
    for c in range(NCORES):
        sl = slice(c * NS, (c + 1) * NS)
        # xt[(q,f), n*GB+gb] = x[q*GB+gb, c*NS+n, f]
        xt = np.ascontiguousarray(
            x[:, sl, :].reshape(4, GB, NS, F).transpose(0, 3, 2, 1)
            .reshape(128, NS * GB)).astype(bf)
        # xe[(q,f), (n,gb,d)] = x[q*GB+gb, idxg[c*NS+n, d], f]
        gath = x[:, idxg[sl], :]                     # [128g, NS, DEG, F]
        xe = np.ascontiguousarray(
            gath.reshape(4, GB, NS, DEG, F).transpose(0, 4, 2, 1, 3)
            .reshape(128, NS * GB * DEG)).astype(bf)
        wha = np.ascontiguousarray(
            Wa[:, sl, :].transpose(2, 1, 0).reshape(H, NS * NA)).astype(bf)
        whv = np.ascontiguousarray(
            Wv[:, sl, :].transpose(2, 1, 0).reshape(H, NS * NV)).astype(e4)
        in_maps.append({"xt": xt, "xe": xe, "whead_adv": wha,
                        "whead_val": whv, **shared})
    return in_maps


def kernel(**inputs) -> np.ndarray:
    global LAST_RESULTS
    from concourse.bass_utils import run_bass_kernel_spmd

    if "nc" not in _CACHE:
        _CACHE["nc"] = _build_program()
    nc = _CACHE["nc"]
    in_maps = _make_in_maps(inputs)
    rr = run_bass_kernel_spmd(nc, in_maps, list(range(NCORES)))
    LAST_RESULTS = rr
    # unshard the K-split: sum the 8 partial head projections, then the
    # tiny dueling tail (~1 MFLOP) on the gathered result
    heads = np.zeros((B, NH), np.float32)
    for c in range(NCORES):
        heads += np.asarray(rr.results[c]["heads"], np.float32)
    b_adv = np.asarray(inputs["b_adv"], np.float32)
    b_v1 = np.asarray(inputs["b_v1"], np.float32)
    W_v2 = np.asarray(inputs["W_v2"], np.float32)
    b_v2 = np.asarray(inputs["b_v2"], np.float32)
    W_v3 = np.asarray(inputs["W_v3"], np.float32)
    b_v3 = np.asarray(inputs["b_v3"], np.float32)
    adv = np.maximum(heads[:, :NA] + b_adv, 0.0).reshape(B, 3, 4)
    val = np.maximum(heads[:, NA:] * (1.0 / VSCALE) + b_v1, 0.0)
    val = np.maximum(val @ W_v2.T + b_v2, 0.0)
    val = val @ W_v3.T + b_v3
    out = val[..., None] + adv - adv.mean(-1, keepdims=True)
    return np.ascontiguousarray(out, np.float32)


# revision 18
# speedup vs baseline: 1.0613x; 1.0613x over previous
"""Trainium2 Bass kernel for nn_BHS_SAGE (GNN message passing + dueling head).

v4: node-parallel K-split. Each NeuronCore owns a 128-node dst slice of ALL
128 graphs (instead of 16 whole graphs). The SAGE layers are node-local
(edges gathered on host into the per-core xe stream, exactly like the
data-parallel baseline), and the huge dueling-head GEMM contracts only this
core's K-slice — so the head weights shrink 8x per core (1.44 MB vs 19.9)
and the head matmuls run at full M=128. One 39 KB ReduceScatter sums the
head partials; each core then runs the tiny dueling tail for its 16 graphs.

Per-core pipeline (128 graphs x 128 dst nodes, F=32, H=128, DEG=16):
  A. z-edge = W_pool @ xe (xe = host-gathered edge-ordered x, bf16,
     [128=(q4,f32), (gb32, n128, d16)]), 512-col psum tiles
  B. DVE reduce_max over d=16 -> aggT [(q,f), (gb, n)], fused bias+relu
  D. ht = relu(W_self x + W_neigh agg + b_sage) -> [128h, (n, g)] bf16
  E. head: ps_adv[128g,12] (bf16 W) / ps_val[128g,64] (fp8-e4m3 W, x8192)
     accumulated over this core's 128 nodes x 128 h
  F. ReduceScatter(add) of [128, 76] -> [16, 76]; dueling tail -> out [16,12]
"""

import numpy as np

B, N, F, H, DEG = 128, 1024, 32, 128, 16
NCORES = 8
NS = N // NCORES          # 128 dst nodes per core
BL = B // NCORES          # 16 graphs per core (output/tail)
NA = 12                   # adv outputs (3 branches x 4 actions)
NV = 64                   # val hidden
NH = NA + NV
GB = B // 4               # 32 graph-blocks of 4 graphs on 128 partitions
VSCALE = 8192.0           # fp8 scale for W_v1

_CACHE = {}
LAST_RESULTS = None


def _build_program():
    import concourse.bacc as bacc
    import concourse.mybir as mybir
    import concourse.tile as tile

    f32 = mybir.dt.float32
    bf16 = mybir.dt.bfloat16
    fp8 = mybir.dt.float8e4
    Relu = mybir.ActivationFunctionType.Relu
    Alu = mybir.AluOpType

    nc = bacc.Bacc("TRN2", target_bir_lowering=False, debug=False,
                   num_devices=NCORES)

    ECOLS = GB * NS * DEG          # 65536 edge columns
    SLAB = 8192                    # edge cols per streamed xe tile
    NSLAB = ECOLS // SLAB          # 8

    # ---- kernel I/O ----
    xt_d = nc.declare_dram_parameter("xt", [128, GB * NS], bf16, isOutput=False)
    xe_d = nc.declare_dram_parameter("xe", [128, ECOLS], bf16, isOutput=False)
    wpool_d = nc.declare_dram_parameter("wpool_bd", [128, 128], bf16, isOutput=False)
    bpool_d = nc.declare_dram_parameter("bpool", [128, 1], f32, isOutput=False)
    wself_d = nc.declare_dram_parameter("wself_bd", [128, 4 * H], bf16, isOutput=False)
    wneigh_d = nc.declare_dram_parameter("wneigh_bd", [128, 4 * H], bf16, isOutput=False)
    bsage_d = nc.declare_dram_parameter("bsage", [128, 1], f32, isOutput=False)
    wha_d = nc.declare_dram_parameter("whead_adv", [128, NS * NA], bf16, isOutput=False)
    whv_d = nc.declare_dram_parameter("whead_val", [128, NS * NV], fp8, isOutput=False)
    heads_d = nc.declare_dram_parameter("heads", [128, NH], f32, isOutput=True)

    import os as _os
    _dbg = _os.environ.get("KDBG") == "1"
    if _dbg:
        dbg_aggT_d = nc.declare_dram_parameter("dbg_aggT", [128, GB * NS], bf16, isOutput=True)
        dbg_ht_d = nc.declare_dram_parameter("dbg_ht", [128, NS * B], bf16, isOutput=True)

    with tile.TileContext(nc) as tc:
        with (
            tc.tile_pool(name="const", bufs=1) as cpool,
            tc.tile_pool(name="big", bufs=1) as bigpool,
        ):
            # ---- constants / persistent tiles ----
            # (only wpool/bpool load up front; the rest queue behind the
            # first xe slab so stage A starts ASAP)
            wpool = cpool.tile([128, 128], bf16)
            nc.sync.dma_start(out=wpool[:], in_=wpool_d[:])
            bpool = cpool.tile([128, 1], f32)
            nc.sync.dma_start(out=bpool[:], in_=bpool_d[:])
            xt = cpool.tile([128, GB * NS], bf16)
            wself = cpool.tile([128, 4 * H], bf16)
            wneigh = cpool.tile([128, 4 * H], bf16)
            bsage = cpool.tile([128, 1], f32)
            wha = cpool.tile([128, NS * NA], bf16)
            whv = cpool.tile([128, NS * NV], fp8)

            ht = bigpool.tile([128, NS * B], bf16)        # [h, n*128+g] 4MB
            aggT = bigpool.tile([128, GB * NS], bf16)     # [(q,f), gb*128+n] 1MB

            # head psums allocated up-front so stage E overlaps stage D
            hd_ps_ctx = tc.tile_pool(name="hd_ps", bufs=1, space="PSUM")
            hd_ps = hd_ps_ctx.__enter__()
            ps_adv = hd_ps.tile([128, NA], f32)
            ps_val = hd_ps.tile([128, NV], f32)

            # ---- stage A+B: aggT = relu(max_d(W_pool @ x[src_d]) + b) ----
            # column order is (n, gb): aggT[:, n*GB+gb]
            with (
                tc.tile_pool(name="xe_sb", bufs=3) as xe_pool,
                tc.tile_pool(name="zb_sb", bufs=3) as zb_pool,
                tc.tile_pool(name="z_ps", bufs=3, space="PSUM") as z_ps,
            ):
                for s in range(NSLAB):
                    xe = xe_pool.tile([128, SLAB], bf16, tag="xe")
                    nc.sync.dma_start(
                        out=xe[:], in_=xe_d[:, s * SLAB:(s + 1) * SLAB])
                    if s == 0:
                        nc.sync.dma_start(out=xt[:], in_=xt_d[:])
                        nc.sync.dma_start(out=wself[:], in_=wself_d[:])
                        nc.sync.dma_start(out=wneigh[:], in_=wneigh_d[:])
                        nc.sync.dma_start(out=bsage[:], in_=bsage_d[:])
                        nc.sync.dma_start(out=wha[:], in_=wha_d[:])
                        nc.sync.dma_start(out=whv[:], in_=whv_d[:])
                    for blk in range(SLAB // 1024):   # 64 nodes x 16 d per blk
                        ps = z_ps.tile([128, 1024], f32, tag="zps")
                        for hf in range(2):           # moving-dim cap is 512
                            nc.tensor.matmul(
                                out=ps[:, hf * 512:(hf + 1) * 512],
                                lhsT=wpool[:],
                                rhs=xe[:, blk * 1024 + hf * 512:
                                        blk * 1024 + (hf + 1) * 512],
                                start=True, stop=True,
                            )
                        t = s * (SLAB // 1024) + blk
                        if t % 5 < 2:
                            # scalar-engine path: psum -> bf16, fast DVE reduce
                            zb = zb_pool.tile([128, 1024], bf16, tag="zb")
                            nc.scalar.copy(out=zb[:], in_=ps[:])
                            nc.vector.reduce_max(
                                out=aggT[:, t * 64:(t + 1) * 64],
                                in_=zb[:].rearrange("p (n d) -> p n d", d=DEG),
                                axis=mybir.AxisListType.X)
                        else:
                            nc.vector.reduce_max(
                                out=aggT[:, t * 64:(t + 1) * 64],
                                in_=ps[:].rearrange("p (n d) -> p n d", d=DEG),
                                axis=mybir.AxisListType.X)
            # fused bias + relu (per-partition bias); monotone max trick
            for h4 in range(4):
                nc.scalar.activation(
                    out=aggT[:, h4 * 1024:(h4 + 1) * 1024],
                    in_=aggT[:, h4 * 1024:(h4 + 1) * 1024],
                    func=Relu, bias=bpool[:])

            if _dbg:
                nc.sync.dma_start(out=dbg_aggT_d[:], in_=aggT[:])

            # ---- stage D: ht = relu(W_self x + W_neigh agg + b_sage) ----
            # n-major layout: ht[h, n*128 + q*32 + gb] -> stage E stationaries
            # are contiguous; D's writes land in 32-elem (64 B) runs
            htv = ht[:].rearrange("p (n qq gb) -> p qq n gb", qq=4, gb=GB)
            with tc.tile_pool(name="h_ps", bufs=2, space="PSUM") as h_ps:
                for q in range(4):
                    for ch in range(GB * NS // 512):   # 8 chunks of 512 cols
                        base = ch * 512
                        ps = h_ps.tile([128, 512], f32, tag="hps")
                        nc.tensor.matmul(
                            out=ps[:],
                            lhsT=wself[:, q * H:(q + 1) * H],
                            rhs=xt[:, base: base + 512],
                            start=True, stop=False)
                        nc.tensor.matmul(
                            out=ps[:],
                            lhsT=wneigh[:, q * H:(q + 1) * H],
                            rhs=aggT[:, base: base + 512],
                            start=False, stop=True)
                        nc.scalar.activation(
                            out=htv[:, q, 16 * ch:16 * ch + 16, :],
                            in_=ps[:].rearrange("p (n gb) -> p n gb", gb=GB),
                            func=Relu, bias=bsage[:])

            if _dbg:
                nc.sync.dma_start(out=dbg_ht_d[:], in_=ht[:])

            # ---- stage E: head psums += ht[:, n-col].T @ W ----
            # ht col block j*128..(j+1)*128 = node j, graphs (q*32+gb) order
            for j in range(NS):
                lhsT = ht[:, j * B:(j + 1) * B]
                nc.tensor.matmul(
                    out=ps_adv[:],
                    lhsT=lhsT,
                    rhs=wha[:, j * NA:(j + 1) * NA],
                    start=(j == 0), stop=(j == NS - 1),
                )
                nc.tensor.matmul(
                    out=ps_val[:],
                    lhsT=lhsT,
                    rhs=whv[:, j * NV:(j + 1) * NV],
                    start=(j == 0), stop=(j == NS - 1),
                )

            # ---- stage F: write partial heads (host sums across cores) ----
            with tc.tile_pool(name="tail", bufs=1) as tp:
                heads = tp.tile([128, NH], f32)
                nc.scalar.copy(out=heads[:, 0:NA], in_=ps_adv[:])
                nc.scalar.copy(out=heads[:, NA:NH], in_=ps_val[:])
                nc.sync.dma_start(out=heads_d[:], in_=heads[:])
            hd_ps_ctx.__exit__(None, None, None)
    nc.compile()
    return nc


def _make_in_maps(inputs):
    import ml_dtypes
    bf = ml_dtypes.bfloat16
    e4 = ml_dtypes.float8_e4m3

    x = np.asarray(inputs["x"], np.float32)
    src = np.asarray(inputs["src"], np.int32)
    W_pool = np.asarray(inputs["W_pool"], np.float32)
    b_pool = np.asarray(inputs["b_pool"], np.float32)
    W_self = np.asarray(inputs["W_self"], np.float32)
    W_neigh = np.asarray(inputs["W_neigh"], np.float32)
    b_sage = np.asarray(inputs["b_sage"], np.float32)
    W_adv = np.asarray(inputs["W_adv"], np.float32)
    b_adv = np.asarray(inputs["b_adv"], np.float32)
    W_v1 = np.asarray(inputs["W_v1"], np.float32)
    b_v1 = np.asarray(inputs["b_v1"], np.float32)
    W_v2 = np.asarray(inputs["W_v2"], np.float32)
    b_v2 = np.asarray(inputs["b_v2"], np.float32)
    W_v3 = np.asarray(inputs["W_v3"], np.float32)
    b_v3 = np.asarray(inputs["b_v3"], np.float32)

    # shared (replicated) tensors
    wpool_bd = np.kron(np.eye(4, dtype=np.float32), W_pool.T)                # [128, 128]
    wpool_bd = np.ascontiguousarray(wpool_bd).astype(bf)
    bpool = np.ascontiguousarray(np.tile(b_pool, 4)[:, None], np.float32)    # [128, 1]
    wself_bd = np.zeros((128, 4 * H), np.float32)                            # [128, 512]
    wneigh_bd = np.zeros((128, 4 * H), np.float32)
    for q in range(4):
        wself_bd[q * 32:(q + 1) * 32, q * H:(q + 1) * H] = W_self.T
        wneigh_bd[q * 32:(q + 1) * 32, q * H:(q + 1) * H] = W_neigh.T
    bsage = np.ascontiguousarray(b_sage[:, None])                            # [128, 1]
    shared = {
        "wpool_bd": wpool_bd, "bpool": bpool,
        "wself_bd": wself_bd.astype(bf), "wneigh_bd": wneigh_bd.astype(bf),
        "bsage": bsage,
    }

    # per-graph src indices (identical across graphs: dgl.batch of one graph)
    idxg = (src[: N * DEG] - 0).reshape(N, DEG)      # graph-0 global indices
    Wa = W_adv.reshape(NA, N, H)                     # [12, n, h]
    Wv = (W_v1 * VSCALE).reshape(NV, N, H)           # [64, n, h]

    in_maps = []
    for c in range(NCORES):
        sl = slice(c * NS, (c + 1) * NS)
        # xt[(q,f), n*GB+gb] = x[q*GB+gb, c*NS+n, f]
        xt = np.ascontiguousarray(
            x[:, sl, :].reshape(4, GB, NS, F).transpose(0, 3, 2, 1)
            .reshape(128, NS * GB)).astype(bf)
        # xe[(q,f), (n,gb,d)] = x[q*GB+gb, idxg[c*NS+n, d], f]
        gath = x[:, idxg[sl], :]                     # [128g, NS, DEG, F]
        xe = np.ascontiguousarray(
            gath.reshape(4, GB, NS, DEG, F).transpose(0, 4, 2, 1, 3)
            .reshape(128, NS * GB * DEG)).astype(bf)
        wha = np.ascontiguousarray(
            Wa[:, sl, :].transpose(2, 1, 0).reshape(H, NS * NA)).astype(bf)
        whv = np.ascontiguousarray(
            Wv[:, sl, :].transpose(2, 1, 0).reshape(H, NS * NV)).astype(e4)
        in_maps.append({"xt": xt, "xe": xe, "whead_adv": wha,
                        "whead_val": whv, **shared})
    return in_maps


def kernel(**inputs) -> np.ndarray:
    global LAST_RESULTS
    from concourse.bass_utils import run_bass_kernel_spmd

    if "nc" not in _CACHE:
        _CACHE["nc"] = _build_program()
    nc = _CACHE["nc"]
    in_maps = _make_in_maps(inputs)
    rr = run_bass_kernel_spmd(nc, in_maps, list(range(NCORES)))
    LAST_RESULTS = rr
    # unshard the K-split: sum the 8 partial head projections, then the
    # tiny dueling tail (~1 MFLOP) on the gathered result
    heads = np.zeros((B, NH), np.float32)
    for c in range(NCORES):
        heads += np.asarray(rr.results[c]["heads"], np.float32)
    b_adv = np.asarray(inputs["b_adv"], np.float32)
    b_v1 = np.asarray(inputs["b_v1"], np.float32)
    W_v2 = np.asarray(inputs["W_v2"], np.float32)
    b_v2 = np.asarray(inputs["b_v2"], np.float32)
    W_v3 = np.asarray(inputs["W_v3"], np.float32)
    b_v3 = np.asarray(inputs["b_v3"], np.float32)
    adv = np.maximum(heads[:, :NA] + b_adv, 0.0).reshape(B, 3, 4)
    val = np.maximum(heads[:, NA:] * (1.0 / VSCALE) + b_v1, 0.0)
    val = np.maximum(val @ W_v2.T + b_v2, 0.0)
    val = val @ W_v3.T + b_v3
    out = val[..., None] + adv - adv.mean(-1, keepdims=True)
    return np.ascontiguousarray(out, np.float32)


# revision 20
# speedup vs baseline: 1.0630x; 1.0016x over previous
"""Trainium2 Bass kernel for nn_BHS_SAGE (GNN message passing + dueling head).

Node-parallel K-split sharding. Each NeuronCore owns a 128-node dst slice
of ALL 128 graphs (instead of 16 whole graphs). The SAGE layers are
node-local (edges gathered on host into the per-core xe stream, like the
data-parallel baseline), and the huge dueling-head GEMM contracts only this
core's K-slice — head weights shrink 8x per core (1.44 MB vs 19.9 MB) and
the head matmuls run at full M=128 instead of M=16. Each core outputs its
partial head projection [128, 76]; the host unshard step sums the 8 K-split
partials and applies the tiny dueling tail (~1 MFLOP) while gathering.
(An on-device ReduceScatter was tried and works, but collectives cost
~120 us on this stack — host-side unshard is the right call here.)

Per-core pipeline (128 graphs x 128 dst nodes, F=32, H=128, DEG=16):
  A. z-edge = W_pool @ xe (xe = host-gathered edge-ordered x, bf16,
     [128=(q4,f32), (n128, gb32, d16)]), 1024-col psum tiles
  B. max over DEG=16: DVE reduce_max from psum (60%) or scalar-engine
     psum->bf16 copy + fast bf16 DVE reduce (40%, engine balancing);
     then fused bias+relu on aggT (monotone-max trick)
  D. ht = relu(W_self x + W_neigh agg + b_sage), n-major [h, n*128+(q,gb)]
     so D's writes are 64 B runs and E's stationaries are contiguous
  E. head: ps_adv[128g,12] (bf16 W) / ps_val[128g,64] (fp8-e4m3 W, x8192
     scale, undone on host) accumulated over 128 nodes x 128 h
  F. partial heads [128, 76] f32 -> DRAM output
"""

import numpy as np

B, N, F, H, DEG = 128, 1024, 32, 128, 16
NCORES = 8
NS = N // NCORES          # 128 dst nodes per core
BL = B // NCORES          # 16 graphs per core (output/tail)
NA = 12                   # adv outputs (3 branches x 4 actions)
NV = 64                   # val hidden
NH = NA + NV
GB = B // 4               # 32 graph-blocks of 4 graphs on 128 partitions
VSCALE = 8192.0           # fp8 scale for W_v1

_CACHE = {}
LAST_RESULTS = None


def _build_program():
    import concourse.bacc as bacc
    import concourse.mybir as mybir
    import concourse.tile as tile

    f32 = mybir.dt.float32
    bf16 = mybir.dt.bfloat16
    fp8 = mybir.dt.float8e4
    Relu = mybir.ActivationFunctionType.Relu

    nc = bacc.Bacc("TRN2", target_bir_lowering=False, debug=False,
                   num_devices=NCORES)

    ECOLS = GB * NS * DEG          # 65536 edge columns
    SLAB = 8192                    # edge cols per streamed xe tile
    NSLAB = ECOLS // SLAB          # 8

    # ---- kernel I/O ----
    xt_d = nc.declare_dram_parameter("xt", [128, GB * NS], bf16, isOutput=False)
    xe_d = nc.declare_dram_parameter("xe", [128, ECOLS], bf16, isOutput=False)
    wpool_d = nc.declare_dram_parameter("wpool_bd", [128, 128], bf16, isOutput=False)
    bpool_d = nc.declare_dram_parameter("bpool", [128, 1], f32, isOutput=False)
    wself_d = nc.declare_dram_parameter("wself_bd", [128, 4 * H], bf16, isOutput=False)
    wneigh_d = nc.declare_dram_parameter("wneigh_bd", [128, 4 * H], bf16, isOutput=False)
    bsage_d = nc.declare_dram_parameter("bsage", [128, 1], f32, isOutput=False)
    wha_d = nc.declare_dram_parameter("whead_adv", [128, NS * NA], bf16, isOutput=False)
    whv_d = nc.declare_dram_parameter("whead_val", [128, NS * NV], fp8, isOutput=False)
    heads_d = nc.declare_dram_parameter("heads", [128, NH], f32, isOutput=True)

    import os as _os
    _dbg = _os.environ.get("KDBG") == "1"
    if _dbg:
        dbg_aggT_d = nc.declare_dram_parameter("dbg_aggT", [128, GB * NS], bf16, isOutput=True)
        dbg_ht_d = nc.declare_dram_parameter("dbg_ht", [128, NS * B], bf16, isOutput=True)

    with tile.TileContext(nc) as tc:
        with (
            tc.tile_pool(name="const", bufs=1) as cpool,
            tc.tile_pool(name="big", bufs=1) as bigpool,
        ):
            # ---- constants / persistent tiles ----
            # (only wpool/bpool load up front; the rest queue behind the
            # first xe slab so stage A starts ASAP)
            wpool = cpool.tile([128, 128], bf16)
            nc.sync.dma_start(out=wpool[:], in_=wpool_d[:])
            bpool = cpool.tile([128, 1], f32)
            nc.sync.dma_start(out=bpool[:], in_=bpool_d[:])
            xt = cpool.tile([128, GB * NS], bf16)
            wself = cpool.tile([128, 4 * H], bf16)
            wneigh = cpool.tile([128, 4 * H], bf16)
            bsage = cpool.tile([128, 1], f32)
            wha = cpool.tile([128, NS * NA], bf16)
            whv = cpool.tile([128, NS * NV], fp8)

            ht = bigpool.tile([128, NS * B], bf16)        # [h, n*128+g] 4MB
            aggT = bigpool.tile([128, GB * NS], bf16)     # [(q,f), gb*128+n] 1MB

            # head psums allocated up-front so stage E overlaps stage D
            hd_ps_ctx = tc.tile_pool(name="hd_ps", bufs=1, space="PSUM")
            hd_ps = hd_ps_ctx.__enter__()
            ps_adv = hd_ps.tile([128, NA], f32)
            ps_val = hd_ps.tile([128, NV], f32)

            # ---- stage A+B: aggT = relu(max_d(W_pool @ x[src_d]) + b) ----
            # column order is (n, gb): aggT[:, n*GB+gb]
            with (
                tc.tile_pool(name="xe_sb", bufs=3) as xe_pool,
                tc.tile_pool(name="zb_sb", bufs=3) as zb_pool,
                tc.tile_pool(name="z_ps", bufs=3, space="PSUM") as z_ps,
            ):
                for s in range(NSLAB):
                    xe = xe_pool.tile([128, SLAB], bf16, tag="xe")
                    nc.sync.dma_start(
                        out=xe[:], in_=xe_d[:, s * SLAB:(s + 1) * SLAB])
                    if s == 0:
                        nc.sync.dma_start(out=xt[:], in_=xt_d[:])
                        nc.sync.dma_start(out=wself[:], in_=wself_d[:])
                        nc.sync.dma_start(out=wneigh[:], in_=wneigh_d[:])
                        nc.sync.dma_start(out=bsage[:], in_=bsage_d[:])
                        nc.sync.dma_start(out=wha[:], in_=wha_d[:])
                        nc.sync.dma_start(out=whv[:], in_=whv_d[:])
                    for blk in range(SLAB // 1024):   # 64 nodes x 16 d per blk
                        ps = z_ps.tile([128, 1024], f32, tag="zps")
                        for hf in range(2):           # moving-dim cap is 512
                            nc.tensor.matmul(
                                out=ps[:, hf * 512:(hf + 1) * 512],
                                lhsT=wpool[:],
                                rhs=xe[:, blk * 1024 + hf * 512:
                                        blk * 1024 + (hf + 1) * 512],
                                start=True, stop=True,
                            )
                        t = s * (SLAB // 1024) + blk
                        if t % 5 < 2:
                            # scalar-engine path: psum -> bf16, fast DVE reduce
                            zb = zb_pool.tile([128, 1024], bf16, tag="zb")
                            nc.scalar.copy(out=zb[:], in_=ps[:])
                            nc.vector.reduce_max(
                                out=aggT[:, t * 64:(t + 1) * 64],
                                in_=zb[:].rearrange("p (n d) -> p n d", d=DEG),
                                axis=mybir.AxisListType.X)
                        else:
                            nc.vector.reduce_max(
                                out=aggT[:, t * 64:(t + 1) * 64],
                                in_=ps[:].rearrange("p (n d) -> p n d", d=DEG),
                                axis=mybir.AxisListType.X)
            # fused bias + relu (per-partition bias); monotone max trick
            for h4 in range(4):
                nc.scalar.activation(
                    out=aggT[:, h4 * 1024:(h4 + 1) * 1024],
                    in_=aggT[:, h4 * 1024:(h4 + 1) * 1024],
                    func=Relu, bias=bpool[:])

            if _dbg:
                nc.sync.dma_start(out=dbg_aggT_d[:], in_=aggT[:])

            # ---- stage D: ht = relu(W_self x + W_neigh agg + b_sage) ----
            # n-major layout: ht[h, n*128 + q*32 + gb] -> stage E stationaries
            # are contiguous; D's writes land in 32-elem (64 B) runs
            htv = ht[:].rearrange("p (n qq gb) -> p qq n gb", qq=4, gb=GB)
            with tc.tile_pool(name="h_ps", bufs=2, space="PSUM") as h_ps:
                for q in range(4):
                    for ch in range(GB * NS // 512):   # 8 chunks of 512 cols
                        base = ch * 512
                        ps = h_ps.tile([128, 512], f32, tag="hps")
                        nc.tensor.matmul(
                            out=ps[:],
                            lhsT=wself[:, q * H:(q + 1) * H],
                            rhs=xt[:, base: base + 512],
                            start=True, stop=False)
                        nc.tensor.matmul(
                            out=ps[:],
                            lhsT=wneigh[:, q * H:(q + 1) * H],
                            rhs=aggT[:, base: base + 512],
                            start=False, stop=True)
                        nc.scalar.activation(
                            out=htv[:, q, 16 * ch:16 * ch + 16, :],
                            in_=ps[:].rearrange("p (n gb) -> p n gb", gb=GB),
                            func=Relu, bias=bsage[:])

            if _dbg:
                nc.sync.dma_start(out=dbg_ht_d[:], in_=ht[:])

            # ---- stage E: head psums += ht[:, n-col].T @ W ----
            # ht col block j*128..(j+1)*128 = node j, graphs (q*32+gb) order
            for j in range(NS):
                lhsT = ht[:, j * B:(j + 1) * B]
                nc.tensor.matmul(
                    out=ps_adv[:],
                    lhsT=lhsT,
                    rhs=wha[:, j * NA:(j + 1) * NA],
                    start=(j == 0), stop=(j == NS - 1),
                )
                nc.tensor.matmul(
                    out=ps_val[:],
                    lhsT=lhsT,
                    rhs=whv[:, j * NV:(j + 1) * NV],
                    start=(j == 0), stop=(j == NS - 1),
                )

            # ---- stage F: write partial heads (host sums across cores) ----
            with tc.tile_pool(name="tail", bufs=1) as tp:
                heads = tp.tile([128, NH], f32)
                nc.scalar.copy(out=heads[:, 0:NA], in_=ps_adv[:])
                nc.scalar.copy(out=heads[:, NA:NH], in_=ps_val[:])
                nc.sync.dma_start(out=heads_d[:], in_=heads[:])
            hd_ps_ctx.__exit__(None, None, None)
    nc.compile()
    return nc


def _make_in_maps(inputs):
    import ml_dtypes
    bf = ml_dtypes.bfloat16
    e4 = ml_dtypes.float8_e4m3

    x = np.asarray(inputs["x"], np.float32)
    src = np.asarray(inputs["src"], np.int32)
    W_pool = np.asarray(inputs["W_pool"], np.float32)
    b_pool = np.asarray(inputs["b_pool"], np.float32)
    W_self = np.asarray(inputs["W_self"], np.float32)
    W_neigh = np.asarray(inputs["W_neigh"], np.float32)
    b_sage = np.asarray(inputs["b_sage"], np.float32)
    W_adv = np.asarray(inputs["W_adv"], np.float32)
    b_adv = np.asarray(inputs["b_adv"], np.float32)
    W_v1 = np.asarray(inputs["W_v1"], np.float32)
    b_v1 = np.asarray(inputs["b_v1"], np.float32)
    W_v2 = np.asarray(inputs["W_v2"], np.float32)
    b_v2 = np.asarray(inputs["b_v2"], np.float32)
    W_v3 = np.asarray(inputs["W_v3"], np.float32)
    b_v3 = np.asarray(inputs["b_v3"], np.float32)

    # shared (replicated) tensors
    wpool_bd = np.kron(np.eye(4, dtype=np.float32), W_pool.T)                # [128, 128]
    wpool_bd = np.ascontiguousarray(wpool_bd).astype(bf)
    bpool = np.ascontiguousarray(np.tile(b_pool, 4)[:, None], np.float32)    # [128, 1]
    wself_bd = np.zeros((128, 4 * H), np.float32)                            # [128, 512]
    wneigh_bd = np.zeros((128, 4 * H), np.float32)
    for q in range(4):
        wself_bd[q * 32:(q + 1) * 32, q * H:(q + 1) * H] = W_self.T
        wneigh_bd[q * 32:(q + 1) * 32, q * H:(q + 1) * H] = W_neigh.T
    bsage = np.ascontiguousarray(b_sage[:, None])                            # [128, 1]
    shared = {
        "wpool_bd": wpool_bd, "bpool": bpool,
        "wself_bd": wself_bd.astype(bf), "wneigh_bd": wneigh_bd.astype(bf),
        "bsage": bsage,
    }

    # per-graph src indices (identical across graphs: dgl.batch of one graph)
    idxg = (src[: N * DEG] - 0).reshape(N, DEG)      # graph-0 global indices
    Wa = W_adv.reshape(NA, N, H)                     # [12, n, h]
    Wv = (W_v1 * VSCALE).reshape(NV, N, H)           # [64, n, h]

    in_maps = []
    for c in range(NCORES):
        sl = slice(c * NS, (c + 1) * NS)
        # xt[(q,f), n*GB+gb] = x[q*GB+gb, c*NS+n, f]
        xt = np.ascontiguousarray(
            x[:, sl, :].reshape(4, GB, NS, F).transpose(0, 3, 2, 1)
            .reshape(128, NS * GB)).astype(bf)
        # xe[(q,f), (n,gb,d)] = x[q*GB+gb, idxg[c*NS+n, d], f]
        gath = x[:, idxg[sl], :]                     # [128g, NS, DEG, F]
        xe = np.ascontiguousarray(
            gath.reshape(4, GB, NS, DEG, F).transpose(0, 4, 2, 1, 3)
            .reshape(128, NS * GB * DEG)).astype(bf)
        wha = np.ascontiguousarray(
            Wa[:, sl, :].transpose(2, 1, 0).reshape(H, NS * NA)).astype(bf)
        whv = np.ascontiguousarray(
            Wv[:, sl, :].transpose(2, 1, 0).reshape(H, NS * NV)).astype(e4)
        in_maps.append({"xt": xt, "xe": xe, "whead_adv": wha,
                        "whead_val": whv, **shared})
    return in_maps


def kernel(**inputs) -> np.ndarray:
    global LAST_RESULTS
    from concourse.bass_utils import run_bass_kernel_spmd

    if "nc" not in _CACHE:
        _CACHE["nc"] = _build_program()
    nc = _CACHE["nc"]
    in_maps = _make_in_maps(inputs)
    rr = run_bass_kernel_spmd(nc, in_maps, list(range(NCORES)))
    LAST_RESULTS = rr
    # unshard the K-split: sum the 8 partial head projections, then the
    # tiny dueling tail (~1 MFLOP) on the gathered result
    heads = np.zeros((B, NH), np.float32)
    for c in range(NCORES):
        heads += np.asarray(rr.results[c]["heads"], np.float32)
    b_adv = np.asarray(inputs["b_adv"], np.float32)
    b_v1 = np.asarray(inputs["b_v1"], np.float32)
    W_v2 = np.asarray(inputs["W_v2"], np.float32)
    b_v2 = np.asarray(inputs["b_v2"], np.float32)
    W_v3 = np.asarray(inputs["W_v3"], np.float32)
    b_v3 = np.asarray(inputs["b_v3"], np.float32)
    adv = np.maximum(heads[:, :NA] + b_adv, 0.0).reshape(B, 3, 4)
    val = np.maximum(heads[:, NA:] * (1.0 / VSCALE) + b_v1, 0.0)
    val = np.maximum(val @ W_v2.T + b_v2, 0.0)
    val = val @ W_v3.T + b_v3
    out = val[..., None] + adv - adv.mean(-1, keepdims=True)
    return np.ascontiguousarray(out, np.float32)


# revision 21
# speedup vs baseline: 1.1338x; 1.0666x over previous
"""Trainium2 Bass kernel for nn_BHS_SAGE (GNN message passing + dueling head).

Node-parallel K-split sharding. Each NeuronCore owns a 128-node dst slice
of ALL 128 graphs (instead of 16 whole graphs). The SAGE layers are
node-local (edges gathered on host into the per-core xe stream, like the
data-parallel baseline), and the huge dueling-head GEMM contracts only this
core's K-slice — head weights shrink 8x per core (1.44 MB vs 19.9 MB) and
the head matmuls run at full M=128 instead of M=16. Each core outputs its
partial head projection [128, 76]; the host unshard step sums the 8 K-split
partials and applies the tiny dueling tail (~1 MFLOP) while gathering.
(An on-device ReduceScatter was tried and works, but collectives cost
~120 us on this stack — host-side unshard is the right call here.)

Per-core pipeline (128 graphs x 128 dst nodes, F=32, H=128, DEG=16):
  A. z-edge = W_pool @ xe (xe = host-gathered edge-ordered x, bf16,
     [128=(q4,f32), (n128, gb32, d16)]), 1024-col psum tiles
  B. max over DEG=16: DVE reduce_max from psum (60%) or scalar-engine
     psum->bf16 copy + fast bf16 DVE reduce (40%, engine balancing);
     then fused bias+relu on aggT (monotone-max trick)
  D. ht = relu(W_self x + W_neigh agg + b_sage), n-major [h, n*128+(q,gb)]
     so D's writes are 64 B runs and E's stationaries are contiguous
  E. head: ps_adv[128g,12] (bf16 W) / ps_val[128g,64] (fp8-e4m3 W, x8192
     scale, undone on host) accumulated over 128 nodes x 128 h
  F. partial heads [128, 76] f32 -> DRAM output
"""

import numpy as np

B, N, F, H, DEG = 128, 1024, 32, 128, 16
NCORES = 8
NS = N // NCORES          # 128 dst nodes per core
BL = B // NCORES          # 16 graphs per core (output/tail)
NA = 12                   # adv outputs (3 branches x 4 actions)
NV = 64                   # val hidden
NH = NA + NV
GB = B // 4               # 32 graph-blocks of 4 graphs on 128 partitions
VSCALE = 8192.0           # fp8 scale for W_v1

_CACHE = {}
LAST_RESULTS = None


def _build_program():
    import concourse.bacc as bacc
    import concourse.mybir as mybir
    import concourse.tile as tile

    f32 = mybir.dt.float32
    bf16 = mybir.dt.bfloat16
    fp8 = mybir.dt.float8e4
    Relu = mybir.ActivationFunctionType.Relu

    nc = bacc.Bacc("TRN2", target_bir_lowering=False, debug=False,
                   num_devices=NCORES)

    ECOLS = GB * NS * DEG          # 65536 edge columns
    SLAB = 8192                    # edge cols per streamed xe tile
    NSLAB = ECOLS // SLAB          # 8

    # ---- kernel I/O ----
    xt_d = nc.declare_dram_parameter("xt", [128, GB * NS], bf16, isOutput=False)
    xe_d = nc.declare_dram_parameter("xe", [128, ECOLS], bf16, isOutput=False)
    wpool_d = nc.declare_dram_parameter("wpool_bd", [128, 128], bf16, isOutput=False)
    bpool_d = nc.declare_dram_parameter("bpool", [128, 1], f32, isOutput=False)
    wself_d = nc.declare_dram_parameter("wself_bd", [128, 4 * H], bf16, isOutput=False)
    wneigh_d = nc.declare_dram_parameter("wneigh_bd", [128, 4 * H], bf16, isOutput=False)
    bsage_d = nc.declare_dram_parameter("bsage", [128, 1], f32, isOutput=False)
    wha_d = nc.declare_dram_parameter("whead_adv", [128, NS * NA], bf16, isOutput=False)
    whv_d = nc.declare_dram_parameter("whead_val", [128, NS * NV], fp8, isOutput=False)
    heads_d = nc.declare_dram_parameter("heads", [128, NH], f32, isOutput=True)

    import os as _os
    _dbg = _os.environ.get("KDBG") == "1"
    if _dbg:
        dbg_aggT_d = nc.declare_dram_parameter("dbg_aggT", [128, GB * NS], bf16, isOutput=True)
        dbg_ht_d = nc.declare_dram_parameter("dbg_ht", [128, NS * B], bf16, isOutput=True)

    with tile.TileContext(nc) as tc:
        with (
            tc.tile_pool(name="const", bufs=1) as cpool,
            tc.tile_pool(name="big", bufs=1) as bigpool,
        ):
            # ---- constants / persistent tiles ----
            # (only wpool/bpool load up front; the rest queue behind the
            # first xe slab so stage A starts ASAP)
            wpool = cpool.tile([128, 128], bf16)
            nc.sync.dma_start(out=wpool[:], in_=wpool_d[:])
            bpool = cpool.tile([128, 1], f32)
            nc.sync.dma_start(out=bpool[:], in_=bpool_d[:])
            xt = cpool.tile([128, GB * NS], bf16)
            wself = cpool.tile([128, 4 * H], bf16)
            wneigh = cpool.tile([128, 4 * H], bf16)
            bsage = cpool.tile([128, 1], f32)
            wha = cpool.tile([128, NS * NA], bf16)
            whv = cpool.tile([128, NS * NV], fp8)

            ht = bigpool.tile([128, NS * B], bf16)        # [h, n*128+g] 4MB
            aggT = bigpool.tile([128, GB * NS], bf16)     # [(q,f), gb*128+n] 1MB

            # head psums allocated up-front so stage E overlaps stage D
            hd_ps_ctx = tc.tile_pool(name="hd_ps", bufs=1, space="PSUM")
            hd_ps = hd_ps_ctx.__enter__()
            ps_adv = hd_ps.tile([128, NA], f32)
            ps_val = hd_ps.tile([128, NV], f32)

            # ---- banded pipeline: per 8192-col xe slab, run stage A
            # (edge matmul), B (max-reduce + bias/relu), D (SAGE combine) and
            # E (head accumulation) for that 16-node band, so PE/DVE/ACT/DMA
            # all overlap across bands. PSUM: z 2x2 + h 2x1 + heads 1 = 7 bk.
            htv = ht[:].rearrange("p (n qq gb) -> p qq n gb", qq=4, gb=GB)
            with (
                tc.tile_pool(name="xe_sb", bufs=3) as xe_pool,
                tc.tile_pool(name="zb_sb", bufs=3) as zb_pool,
                tc.tile_pool(name="z_ps", bufs=2, space="PSUM") as z_ps,
                tc.tile_pool(name="h_ps", bufs=2, space="PSUM") as h_ps,
            ):
                for s in range(NSLAB):                # band = 16 dst nodes
                    xe = xe_pool.tile([128, SLAB], bf16, tag="xe")
                    nc.sync.dma_start(
                        out=xe[:], in_=xe_d[:, s * SLAB:(s + 1) * SLAB])
                    if s == 0:
                        nc.sync.dma_start(out=xt[:], in_=xt_d[:])
                        nc.sync.dma_start(out=wself[:], in_=wself_d[:])
                        nc.sync.dma_start(out=wneigh[:], in_=wneigh_d[:])
                        nc.sync.dma_start(out=bsage[:], in_=bsage_d[:])
                        nc.sync.dma_start(out=wha[:], in_=wha_d[:])
                        nc.sync.dma_start(out=whv[:], in_=whv_d[:])
                    # A+B: 8 tiles of 1024 edge cols -> aggT[:, s*512:(s+1)*512]
                    for blk in range(SLAB // 1024):
                        ps = z_ps.tile([128, 1024], f32, tag="zps")
                        for hf in range(2):           # moving-dim cap is 512
                            nc.tensor.matmul(
                                out=ps[:, hf * 512:(hf + 1) * 512],
                                lhsT=wpool[:],
                                rhs=xe[:, blk * 1024 + hf * 512:
                                        blk * 1024 + (hf + 1) * 512],
                                start=True, stop=True,
                            )
                        t = s * (SLAB // 1024) + blk
                        if t % 5 < 2:
                            # scalar-engine path: psum -> bf16, fast DVE reduce
                            zb = zb_pool.tile([128, 1024], bf16, tag="zb")
                            nc.scalar.copy(out=zb[:], in_=ps[:])
                            nc.vector.reduce_max(
                                out=aggT[:, t * 64:(t + 1) * 64],
                                in_=zb[:].rearrange("p (n d) -> p n d", d=DEG),
                                axis=mybir.AxisListType.X)
                        else:
                            nc.vector.reduce_max(
                                out=aggT[:, t * 64:(t + 1) * 64],
                                in_=ps[:].rearrange("p (n d) -> p n d", d=DEG),
                                axis=mybir.AxisListType.X)
                    # B tail: fused bias + relu on this band (monotone max)
                    nc.scalar.activation(
                        out=aggT[:, s * 512:(s + 1) * 512],
                        in_=aggT[:, s * 512:(s + 1) * 512],
                        func=Relu, bias=bpool[:])
                    # D: ht for this band's 16 nodes (all 4 quadrants)
                    base = s * 512
                    for q in range(4):
                        psh = h_ps.tile([128, 512], f32, tag="hps")
                        nc.tensor.matmul(
                            out=psh[:],
                            lhsT=wself[:, q * H:(q + 1) * H],
                            rhs=xt[:, base: base + 512],
                            start=True, stop=False)
                        nc.tensor.matmul(
                            out=psh[:],
                            lhsT=wneigh[:, q * H:(q + 1) * H],
                            rhs=aggT[:, base: base + 512],
                            start=False, stop=True)
                        nc.scalar.activation(
                            out=htv[:, q, 16 * s:16 * s + 16, :],
                            in_=psh[:].rearrange("p (n gb) -> p n gb", gb=GB),
                            func=Relu, bias=bsage[:])
                    # E: accumulate head psums for this band's nodes
                    for j in range(16 * s, 16 * s + 16):
                        lhsT = ht[:, j * B:(j + 1) * B]
                        nc.tensor.matmul(
                            out=ps_adv[:],
                            lhsT=lhsT,
                            rhs=wha[:, j * NA:(j + 1) * NA],
                            start=(j == 0), stop=(j == NS - 1),
                        )
                        nc.tensor.matmul(
                            out=ps_val[:],
                            lhsT=lhsT,
                            rhs=whv[:, j * NV:(j + 1) * NV],
                            start=(j == 0), stop=(j == NS - 1),
                        )

            if _dbg:
                nc.sync.dma_start(out=dbg_aggT_d[:], in_=aggT[:])
            if _dbg:
                nc.sync.dma_start(out=dbg_ht_d[:], in_=ht[:])

            # ---- stage F: write partial heads (host sums across cores) ----
            with tc.tile_pool(name="tail", bufs=1) as tp:
                heads = tp.tile([128, NH], f32)
                nc.scalar.copy(out=heads[:, 0:NA], in_=ps_adv[:])
                nc.scalar.copy(out=heads[:, NA:NH], in_=ps_val[:])
                nc.sync.dma_start(out=heads_d[:], in_=heads[:])
            hd_ps_ctx.__exit__(None, None, None)
    nc.compile()
    return nc


def _make_in_maps(inputs):
    import ml_dtypes
    bf = ml_dtypes.bfloat16
    e4 = ml_dtypes.float8_e4m3

    x = np.asarray(inputs["x"], np.float32)
    src = np.asarray(inputs["src"], np.int32)
    W_pool = np.asarray(inputs["W_pool"], np.float32)
    b_pool = np.asarray(inputs["b_pool"], np.float32)
    W_self = np.asarray(inputs["W_self"], np.float32)
    W_neigh = np.asarray(inputs["W_neigh"], np.float32)
    b_sage = np.asarray(inputs["b_sage"], np.float32)
    W_adv = np.asarray(inputs["W_adv"], np.float32)
    b_adv = np.asarray(inputs["b_adv"], np.float32)
    W_v1 = np.asarray(inputs["W_v1"], np.float32)
    b_v1 = np.asarray(inputs["b_v1"], np.float32)
    W_v2 = np.asarray(inputs["W_v2"], np.float32)
    b_v2 = np.asarray(inputs["b_v2"], np.float32)
    W_v3 = np.asarray(inputs["W_v3"], np.float32)
    b_v3 = np.asarray(inputs["b_v3"], np.float32)

    # shared (replicated) tensors
    wpool_bd = np.kron(np.eye(4, dtype=np.float32), W_pool.T)                # [128, 128]
    wpool_bd = np.ascontiguousarray(wpool_bd).astype(bf)
    bpool = np.ascontiguousarray(np.tile(b_pool, 4)[:, None], np.float32)    # [128, 1]
    wself_bd = np.zeros((128, 4 * H), np.float32)                            # [128, 512]
    wneigh_bd = np.zeros((128, 4 * H), np.float32)
    for q in range(4):
        wself_bd[q * 32:(q + 1) * 32, q * H:(q + 1) * H] = W_self.T
        wneigh_bd[q * 32:(q + 1) * 32, q * H:(q + 1) * H] = W_neigh.T
    bsage = np.ascontiguousarray(b_sage[:, None])                            # [128, 1]
    shared = {
        "wpool_bd": wpool_bd, "bpool": bpool,
        "wself_bd": wself_bd.astype(bf), "wneigh_bd": wneigh_bd.astype(bf),
        "bsage": bsage,
    }

    # per-graph src indices (identical across graphs: dgl.batch of one graph)
    idxg = (src[: N * DEG] - 0).reshape(N, DEG)      # graph-0 global indices
    Wa = W_adv.reshape(NA, N, H)                     # [12, n, h]
    Wv = (W_v1 * VSCALE).reshape(NV, N, H)           # [64, n, h]

    in_maps = []
    for c in range(NCORES):
        sl = slice(c * NS, (c + 1) * NS)
        # xt[(q,f), n*GB+gb] = x[q*GB+gb, c*NS+n, f]
        xt = np.ascontiguousarray(
            x[:, sl, :].reshape(4, GB, NS, F).transpose(0, 3, 2, 1)
            .reshape(128, NS * GB)).astype(bf)
        # xe[(q,f), (n,gb,d)] = x[q*GB+gb, idxg[c*NS+n, d], f]
        gath = x[:, idxg[sl], :]                     # [128g, NS, DEG, F]
        xe = np.ascontiguousarray(
            gath.reshape(4, GB, NS, DEG, F).transpose(0, 4, 2, 1, 3)
            .reshape(128, NS * GB * DEG)).astype(bf)
        wha = np.ascontiguousarray(
            Wa[:, sl, :].transpose(2, 1, 0).reshape(H, NS * NA)).astype(bf)
        whv = np.ascontiguousarray(
            Wv[:, sl, :].transpose(2, 1, 0).reshape(H, NS * NV)).astype(e4)
        in_maps.append({"xt": xt, "xe": xe, "whead_adv": wha,
                        "whead_val": whv, **shared})
    return in_maps


def kernel(**inputs) -> np.ndarray:
    global LAST_RESULTS
    from concourse.bass_utils import run_bass_kernel_spmd

    if "nc" not in _CACHE:
        _CACHE["nc"] = _build_program()
    nc = _CACHE["nc"]
    in_maps = _make_in_maps(inputs)
    rr = run_bass_kernel_spmd(nc, in_maps, list(range(NCORES)))
    LAST_RESULTS = rr
    # unshard the K-split: sum the 8 partial head projections, then the
    # tiny dueling tail (~1 MFLOP) on the gathered result
    heads = np.zeros((B, NH), np.float32)
    for c in range(NCORES):
        heads += np.asarray(rr.results[c]["heads"], np.float32)
    b_adv = np.asarray(inputs["b_adv"], np.float32)
    b_v1 = np.asarray(inputs["b_v1"], np.float32)
    W_v2 = np.asarray(inputs["W_v2"], np.float32)
    b_v2 = np.asarray(inputs["b_v2"], np.float32)
    W_v3 = np.asarray(inputs["W_v3"], np.float32)
    b_v3 = np.asarray(inputs["b_v3"], np.float32)
    adv = np.maximum(heads[:, :NA] + b_adv, 0.0).reshape(B, 3, 4)
    val = np.maximum(heads[:, NA:] * (1.0 / VSCALE) + b_v1, 0.0)
    val = np.maximum(val @ W_v2.T + b_v2, 0.0)
    val = val @ W_v3.T + b_v3
    out = val[..., None] + adv - adv.mean(-1, keepdims=True)
    return np.ascontiguousarray(out, np.float32)


# revision 22
# speedup vs baseline: 1.2340x; 1.0884x over previous
"""Trainium2 Bass kernel for nn_BHS_SAGE (GNN message passing + dueling head).

Node-parallel K-split sharding. Each NeuronCore owns a 128-node dst slice
of ALL 128 graphs (instead of 16 whole graphs). The SAGE layers are
node-local (edges gathered on host into the per-core xe stream, like the
data-parallel baseline), and the huge dueling-head GEMM contracts only this
core's K-slice — head weights shrink 8x per core (1.44 MB vs 19.9 MB) and
the head matmuls run at full M=128 instead of M=16. Each core outputs its
partial head projection [128, 76]; the host unshard step sums the 8 K-split
partials and applies the tiny dueling tail (~1 MFLOP) while gathering.
(An on-device ReduceScatter was tried and works, but collectives cost
~120 us on this stack — host-side unshard is the right call here.)

Per-core pipeline (128 graphs x 128 dst nodes, F=32, H=128, DEG=16):
  A. z-edge = W_pool @ xe (xe = host-gathered edge-ordered x, bf16,
     [128=(q4,f32), (n128, gb32, d16)]), 1024-col psum tiles
  B. max over DEG=16: DVE reduce_max from psum (60%) or scalar-engine
     psum->bf16 copy + fast bf16 DVE reduce (40%, engine balancing);
     then fused bias+relu on aggT (monotone-max trick)
  D. ht = relu(W_self x + W_neigh agg + b_sage), n-major [h, n*128+(q,gb)]
     so D's writes are 64 B runs and E's stationaries are contiguous
  E. head: ps_adv[128g,12] (bf16 W) / ps_val[128g,64] (fp8-e4m3 W, x8192
     scale, undone on host) accumulated over 128 nodes x 128 h
  F. partial heads [128, 76] f32 -> DRAM output
"""

import numpy as np

B, N, F, H, DEG = 128, 1024, 32, 128, 16
NCORES = 8
NS = N // NCORES          # 128 dst nodes per core
BL = B // NCORES          # 16 graphs per core (output/tail)
NA = 12                   # adv outputs (3 branches x 4 actions)
NV = 64                   # val hidden
NH = NA + NV
GB = B // 4               # 32 graph-blocks of 4 graphs on 128 partitions
VSCALE = 8192.0           # fp8 scale for W_v1

_CACHE = {}
LAST_RESULTS = None


def _build_program():
    import concourse.bacc as bacc
    import concourse.mybir as mybir
    import concourse.tile as tile

    f32 = mybir.dt.float32
    bf16 = mybir.dt.bfloat16
    fp8 = mybir.dt.float8e4
    Relu = mybir.ActivationFunctionType.Relu

    nc = bacc.Bacc("TRN2", target_bir_lowering=False, debug=False,
                   num_devices=NCORES)

    ECOLS = GB * NS * DEG          # 65536 edge columns
    SLAB = 8192                    # edge cols per streamed xe tile
    NSLAB = ECOLS // SLAB          # 8

    # ---- kernel I/O ----
    xt_d = nc.declare_dram_parameter("xt", [128, GB * NS], bf16, isOutput=False)
    xe_d = nc.declare_dram_parameter("xe", [128, ECOLS], bf16, isOutput=False)
    wpool_d = nc.declare_dram_parameter("wpool_bd", [128, 128], bf16, isOutput=False)
    bpool_d = nc.declare_dram_parameter("bpool", [128, 1], f32, isOutput=False)
    wself_d = nc.declare_dram_parameter("wself_bd", [128, 4 * H], bf16, isOutput=False)
    wneigh_d = nc.declare_dram_parameter("wneigh_bd", [128, 4 * H], bf16, isOutput=False)
    bsage_d = nc.declare_dram_parameter("bsage", [128, 1], f32, isOutput=False)
    wha_d = nc.declare_dram_parameter("whead_adv", [128, NS * NA], bf16, isOutput=False)
    whv_d = nc.declare_dram_parameter("whead_val", [128, NS * NV], fp8, isOutput=False)
    heads_d = nc.declare_dram_parameter("heads", [128, NH], f32, isOutput=True)

    import os as _os
    _dbg = _os.environ.get("KDBG") == "1"
    if _dbg:
        dbg_aggT_d = nc.declare_dram_parameter("dbg_aggT", [128, GB * NS], bf16, isOutput=True)
        dbg_ht_d = nc.declare_dram_parameter("dbg_ht", [128, NS * B], bf16, isOutput=True)

    with tile.TileContext(nc) as tc:
        with (
            tc.tile_pool(name="const", bufs=1) as cpool,
            tc.tile_pool(name="big", bufs=1) as bigpool,
        ):
            # ---- constants / persistent tiles ----
            # (only wpool/bpool load up front; the rest queue behind the
            # first xe slab so stage A starts ASAP)
            wpool = cpool.tile([128, 128], bf16)
            nc.sync.dma_start(out=wpool[:], in_=wpool_d[:])
            bpool = cpool.tile([128, 1], f32)
            nc.sync.dma_start(out=bpool[:], in_=bpool_d[:])
            xt = cpool.tile([128, GB * NS], bf16)
            wself = cpool.tile([128, 4 * H], bf16)
            wneigh = cpool.tile([128, 4 * H], bf16)
            bsage = cpool.tile([128, 1], f32)
            wha = cpool.tile([128, NS * NA], bf16)
            whv = cpool.tile([128, NS * NV], fp8)

            ht = bigpool.tile([128, NS * B], bf16)        # [h, n*128+g] 4MB
            aggT = bigpool.tile([128, GB * NS], bf16)     # [(q,f), gb*128+n] 1MB

            # head psums allocated up-front so stage E overlaps stage D
            hd_ps_ctx = tc.tile_pool(name="hd_ps", bufs=1, space="PSUM")
            hd_ps = hd_ps_ctx.__enter__()
            ps_adv = hd_ps.tile([128, NA], f32)
            ps_val = hd_ps.tile([128, NV], f32)

            # ---- banded pipeline: per 8192-col xe slab, run stage A
            # (edge matmul), B (max-reduce + bias/relu), D (SAGE combine) and
            # E (head accumulation) for that 16-node band, so PE/DVE/ACT/DMA
            # all overlap across bands. PSUM: z 2x2 + h 2x1 + heads 1 = 7 bk.
            htv = ht[:].rearrange("p (n qq gb) -> p qq n gb", qq=4, gb=GB)
            with (
                tc.tile_pool(name="xe_sb", bufs=4) as xe_pool,
                tc.tile_pool(name="zb_sb", bufs=3) as zb_pool,
                tc.tile_pool(name="z_ps", bufs=2, space="PSUM") as z_ps,
                tc.tile_pool(name="h_ps", bufs=2, space="PSUM") as h_ps,
            ):
                for s in range(NSLAB):                # band = 16 dst nodes
                    xe = xe_pool.tile([128, SLAB], bf16, tag="xe")
                    nc.sync.dma_start(
                        out=xe[:], in_=xe_d[:, s * SLAB:(s + 1) * SLAB])
                    if s == 0:
                        nc.sync.dma_start(out=xt[:], in_=xt_d[:])
                        nc.sync.dma_start(out=wself[:], in_=wself_d[:])
                        nc.sync.dma_start(out=wneigh[:], in_=wneigh_d[:])
                        nc.sync.dma_start(out=bsage[:], in_=bsage_d[:])
                        nc.sync.dma_start(out=wha[:], in_=wha_d[:])
                        nc.sync.dma_start(out=whv[:], in_=whv_d[:])
                    # A+B: 8 tiles of 1024 edge cols -> aggT[:, s*512:(s+1)*512]
                    for blk in range(SLAB // 1024):
                        ps = z_ps.tile([128, 1024], f32, tag="zps")
                        for hf in range(2):           # moving-dim cap is 512
                            nc.tensor.matmul(
                                out=ps[:, hf * 512:(hf + 1) * 512],
                                lhsT=wpool[:],
                                rhs=xe[:, blk * 1024 + hf * 512:
                                        blk * 1024 + (hf + 1) * 512],
                                start=True, stop=True,
                            )
                        t = s * (SLAB // 1024) + blk
                        if t % 2 == 0:
                            # scalar-engine path: psum -> bf16, fast DVE reduce
                            zb = zb_pool.tile([128, 1024], bf16, tag="zb")
                            nc.scalar.copy(out=zb[:], in_=ps[:])
                            nc.vector.reduce_max(
                                out=aggT[:, t * 64:(t + 1) * 64],
                                in_=zb[:].rearrange("p (n d) -> p n d", d=DEG),
                                axis=mybir.AxisListType.X)
                        else:
                            nc.vector.reduce_max(
                                out=aggT[:, t * 64:(t + 1) * 64],
                                in_=ps[:].rearrange("p (n d) -> p n d", d=DEG),
                                axis=mybir.AxisListType.X)
                    # B tail: fused bias + relu on this band (monotone max)
                    nc.scalar.activation(
                        out=aggT[:, s * 512:(s + 1) * 512],
                        in_=aggT[:, s * 512:(s + 1) * 512],
                        func=Relu, bias=bpool[:])
                    # D: ht for this band's 16 nodes (all 4 quadrants)
                    base = s * 512
                    for q in range(4):
                        psh = h_ps.tile([128, 512], f32, tag="hps")
                        nc.tensor.matmul(
                            out=psh[:],
                            lhsT=wself[:, q * H:(q + 1) * H],
                            rhs=xt[:, base: base + 512],
                            start=True, stop=False)
                        nc.tensor.matmul(
                            out=psh[:],
                            lhsT=wneigh[:, q * H:(q + 1) * H],
                            rhs=aggT[:, base: base + 512],
                            start=False, stop=True)
                        nc.scalar.activation(
                            out=htv[:, q, 16 * s:16 * s + 16, :],
                            in_=psh[:].rearrange("p (n gb) -> p n gb", gb=GB),
                            func=Relu, bias=bsage[:])
                    # E: accumulate head psums for this band's nodes
                    for j in range(16 * s, 16 * s + 16):
                        lhsT = ht[:, j * B:(j + 1) * B]
                        nc.tensor.matmul(
                            out=ps_adv[:],
                            lhsT=lhsT,
                            rhs=wha[:, j * NA:(j + 1) * NA],
                            start=(j == 0), stop=(j == NS - 1),
                        )
                        nc.tensor.matmul(
                            out=ps_val[:],
                            lhsT=lhsT,
                            rhs=whv[:, j * NV:(j + 1) * NV],
                            start=(j == 0), stop=(j == NS - 1),
                        )

            if _dbg:
                nc.sync.dma_start(out=dbg_aggT_d[:], in_=aggT[:])
            if _dbg:
                nc.sync.dma_start(out=dbg_ht_d[:], in_=ht[:])

            # ---- stage F: write partial heads (host sums across cores) ----
            with tc.tile_pool(name="tail", bufs=1) as tp:
                heads = tp.tile([128, NH], f32)
                nc.scalar.copy(out=heads[:, 0:NA], in_=ps_adv[:])
                nc.scalar.copy(out=heads[:, NA:NH], in_=ps_val[:])
                nc.sync.dma_start(out=heads_d[:], in_=heads[:])
            hd_ps_ctx.__exit__(None, None, None)
    nc.compile()
    return nc


def _make_in_maps(inputs):
    import ml_dtypes
    bf = ml_dtypes.bfloat16
    e4 = ml_dtypes.float8_e4m3

    x = np.asarray(inputs["x"], np.float32)
    src = np.asarray(inputs["src"], np.int32)
    W_pool = np.asarray(inputs["W_pool"], np.float32)
    b_pool = np.asarray(inputs["b_pool"], np.float32)
    W_self = np.asarray(inputs["W_self"], np.float32)
    W_neigh = np.asarray(inputs["W_neigh"], np.float32)
    b_sage = np.asarray(inputs["b_sage"], np.float32)
    W_adv = np.asarray(inputs["W_adv"], np.float32)
    b_adv = np.asarray(inputs["b_adv"], np.float32)
    W_v1 = np.asarray(inputs["W_v1"], np.float32)
    b_v1 = np.asarray(inputs["b_v1"], np.float32)
    W_v2 = np.asarray(inputs["W_v2"], np.float32)
    b_v2 = np.asarray(inputs["b_v2"], np.float32)
    W_v3 = np.asarray(inputs["W_v3"], np.float32)
    b_v3 = np.asarray(inputs["b_v3"], np.float32)

    # shared (replicated) tensors
    wpool_bd = np.kron(np.eye(4, dtype=np.float32), W_pool.T)                # [128, 128]
    wpool_bd = np.ascontiguousarray(wpool_bd).astype(bf)
    bpool = np.ascontiguousarray(np.tile(b_pool, 4)[:, None], np.float32)    # [128, 1]
    wself_bd = np.zeros((128, 4 * H), np.float32)                            # [128, 512]
    wneigh_bd = np.zeros((128, 4 * H), np.float32)
    for q in range(4):
        wself_bd[q * 32:(q + 1) * 32, q * H:(q + 1) * H] = W_self.T
        wneigh_bd[q * 32:(q + 1) * 32, q * H:(q + 1) * H] = W_neigh.T
    bsage = np.ascontiguousarray(b_sage[:, None])                            # [128, 1]
    shared = {
        "wpool_bd": wpool_bd, "bpool": bpool,
        "wself_bd": wself_bd.astype(bf), "wneigh_bd": wneigh_bd.astype(bf),
        "bsage": bsage,
    }

    # per-graph src indices (identical across graphs: dgl.batch of one graph)
    idxg = (src[: N * DEG] - 0).reshape(N, DEG)      # graph-0 global indices
    Wa = W_adv.reshape(NA, N, H)                     # [12, n, h]
    Wv = (W_v1 * VSCALE).reshape(NV, N, H)           # [64, n, h]

    in_maps = []
    for c in range(NCORES):
        sl = slice(c * NS, (c + 1) * NS)
        # xt[(q,f), n*GB+gb] = x[q*GB+gb, c*NS+n, f]
        xt = np.ascontiguousarray(
            x[:, sl, :].reshape(4, GB, NS, F).transpose(0, 3, 2, 1)
            .reshape(128, NS * GB)).astype(bf)
        # xe[(q,f), (n,gb,d)] = x[q*GB+gb, idxg[c*NS+n, d], f]
        gath = x[:, idxg[sl], :]                     # [128g, NS, DEG, F]
        xe = np.ascontiguousarray(
            gath.reshape(4, GB, NS, DEG, F).transpose(0, 4, 2, 1, 3)
            .reshape(128, NS * GB * DEG)).astype(bf)
        wha = np.ascontiguousarray(
            Wa[:, sl, :].transpose(2, 1, 0).reshape(H, NS * NA)).astype(bf)
        whv = np.ascontiguousarray(
            Wv[:, sl, :].transpose(2, 1, 0).reshape(H, NS * NV)).astype(e4)
        in_maps.append({"xt": xt, "xe": xe, "whead_adv": wha,
                        "whead_val": whv, **shared})
    return in_maps


def kernel(**inputs) -> np.ndarray:
    global LAST_RESULTS
    from concourse.bass_utils import run_bass_kernel_spmd

    if "nc" not in _CACHE:
        _CACHE["nc"] = _build_program()
    nc = _CACHE["nc"]
    in_maps = _make_in_maps(inputs)
    rr = run_bass_kernel_spmd(nc, in_maps, list(range(NCORES)))
    LAST_RESULTS = rr
    # unshard the K-split: sum the 8 partial head projections, then the
    # tiny dueling tail (~1 MFLOP) on the gathered result
    heads = np.zeros((B, NH), np.float32)
    for c in range(NCORES):
        heads += np.asarray(rr.results[c]["heads"], np.float32)
    b_adv = np.asarray(inputs["b_adv"], np.float32)
    b_v1 = np.asarray(inputs["b_v1"], np.float32)
    W_v2 = np.asarray(inputs["W_v2"], np.float32)
    b_v2 = np.asarray(inputs["b_v2"], np.float32)
    W_v3 = np.asarray(inputs["W_v3"], np.float32)
    b_v3 = np.asarray(inputs["b_v3"], np.float32)
    adv = np.maximum(heads[:, :NA] + b_adv, 0.0).reshape(B, 3, 4)
    val = np.maximum(heads[:, NA:] * (1.0 / VSCALE) + b_v1, 0.0)
    val = np.maximum(val @ W_v2.T + b_v2, 0.0)
    val = val @ W_v3.T + b_v3
    out = val[..., None] + adv - adv.mean(-1, keepdims=True)
    return np.ascontiguousarray(out, np.float32)


# revision 24
# speedup vs baseline: 1.2771x; 1.0349x over previous
"""Trainium2 Bass kernel for nn_BHS_SAGE (GNN message passing + dueling head).

Node-parallel K-split sharding. Each NeuronCore owns a 128-node dst slice
of ALL 128 graphs (instead of 16 whole graphs). The SAGE layers are
node-local (edges gathered on host into the per-core xe stream, like the
data-parallel baseline), and the huge dueling-head GEMM contracts only this
core's K-slice — head weights shrink 8x per core (1.44 MB vs 19.9 MB) and
the head matmuls run at full M=128 instead of M=16. Each core outputs its
partial head projection [128, 76]; the host unshard step sums the 8 K-split
partials and applies the tiny dueling tail (~1 MFLOP) while gathering.
(An on-device ReduceScatter was tried and works, but collectives cost
~120 us on this stack — host-side unshard is the right call here.)

Per-core pipeline (128 graphs x 128 dst nodes, F=32, H=128, DEG=16):
  A. z-edge = W_pool @ xe (xe = host-gathered edge-ordered x, bf16,
     [128=(q4,f32), (n128, gb32, d16)]), 1024-col psum tiles
  B. max over DEG=16: DVE reduce_max from psum (60%) or scalar-engine
     psum->bf16 copy + fast bf16 DVE reduce (40%, engine balancing);
     then fused bias+relu on aggT (monotone-max trick)
  D. ht = relu(W_self x + W_neigh agg + b_sage), n-major [h, n*128+(q,gb)]
     so D's writes are 64 B runs and E's stationaries are contiguous
  E. head: ps_adv[128g,12] (bf16 W) / ps_val[128g,64] (fp8-e4m3 W, x8192
     scale, undone on host) accumulated over 128 nodes x 128 h
  F. partial heads [128, 76] f32 -> DRAM output
"""

import numpy as np

B, N, F, H, DEG = 128, 1024, 32, 128, 16
NCORES = 8
NS = N // NCORES          # 128 dst nodes per core
BL = B // NCORES          # 16 graphs per core (output/tail)
NA = 12                   # adv outputs (3 branches x 4 actions)
NV = 64                   # val hidden
NH = NA + NV
GB = B // 4               # 32 graph-blocks of 4 graphs on 128 partitions
VSCALE = 8192.0           # fp8 scale for W_v1

_CACHE = {}
LAST_RESULTS = None


def _build_program():
    import concourse.bacc as bacc
    import concourse.mybir as mybir
    import concourse.tile as tile

    f32 = mybir.dt.float32
    bf16 = mybir.dt.bfloat16
    fp8 = mybir.dt.float8e4
    Relu = mybir.ActivationFunctionType.Relu

    nc = bacc.Bacc("TRN2", target_bir_lowering=False, debug=False,
                   num_devices=NCORES)

    ECOLS = GB * NS * DEG          # 65536 edge columns
    SLAB = 8192                    # edge cols per streamed xe tile
    NSLAB = ECOLS // SLAB          # 8

    # ---- kernel I/O ----
    xt_d = nc.declare_dram_parameter("xt", [128, GB * NS], bf16, isOutput=False)
    xe_d = nc.declare_dram_parameter("xe", [128, ECOLS], bf16, isOutput=False)
    wpool_d = nc.declare_dram_parameter("wpool_bd", [128, 128], bf16, isOutput=False)
    bpool_d = nc.declare_dram_parameter("bpool", [128, 1], f32, isOutput=False)
    wself_d = nc.declare_dram_parameter("wself_bd", [128, 4 * H], bf16, isOutput=False)
    wneigh_d = nc.declare_dram_parameter("wneigh_bd", [128, 4 * H], bf16, isOutput=False)
    bsage_d = nc.declare_dram_parameter("bsage", [128, 1], f32, isOutput=False)
    wha_d = nc.declare_dram_parameter("whead_adv", [128, NS * NA], bf16, isOutput=False)
    whv_d = nc.declare_dram_parameter("whead_val", [128, NS * NV], fp8, isOutput=False)
    heads_d = nc.declare_dram_parameter("heads", [128, NH], f32, isOutput=True)

    import os as _os
    _dbg = _os.environ.get("KDBG") == "1"
    if _dbg:
        dbg_aggT_d = nc.declare_dram_parameter("dbg_aggT", [128, GB * NS], bf16, isOutput=True)
        dbg_ht_d = nc.declare_dram_parameter("dbg_ht", [128, NS * B], bf16, isOutput=True)

    with tile.TileContext(nc) as tc:
        with (
            tc.tile_pool(name="const", bufs=1) as cpool,
            tc.tile_pool(name="big", bufs=1) as bigpool,
        ):
            # ---- constants / persistent tiles ----
            # (only wpool/bpool load up front; the rest queue behind the
            # first xe slab so stage A starts ASAP)
            wpool = cpool.tile([128, 128], bf16)
            nc.sync.dma_start(out=wpool[:], in_=wpool_d[:])
            bpool = cpool.tile([128, 1], f32)
            nc.sync.dma_start(out=bpool[:], in_=bpool_d[:])
            xt = cpool.tile([128, GB * NS], bf16)
            wself = cpool.tile([128, 4 * H], bf16)
            wneigh = cpool.tile([128, 4 * H], bf16)
            bsage = cpool.tile([128, 1], f32)
            wha = cpool.tile([128, NS * NA], bf16)
            whv = cpool.tile([128, NS * NV], fp8)

            ht = bigpool.tile([128, NS * B], bf16)        # [h, n*128+g] 4MB
            aggT = bigpool.tile([128, GB * NS], bf16)     # [(q,f), gb*128+n] 1MB

            # head psums allocated up-front so stage E overlaps stage D
            hd_ps_ctx = tc.tile_pool(name="hd_ps", bufs=1, space="PSUM")
            hd_ps = hd_ps_ctx.__enter__()
            ps_adv = hd_ps.tile([128, NA], f32)
            ps_val = hd_ps.tile([128, NV], f32)

            # ---- banded pipeline: per 8192-col xe slab, run stage A
            # (edge matmul), B (max-reduce + bias/relu), D (SAGE combine) and
            # E (head accumulation) for that 16-node band, so PE/DVE/ACT/DMA
            # all overlap across bands. PSUM: z 2x2 + h 2x1 + heads 1 = 7 bk.
            htv = ht[:].rearrange("p (n qq gb) -> p qq n gb", qq=4, gb=GB)
            with (
                tc.tile_pool(name="xe_sb", bufs=4) as xe_pool,
                tc.tile_pool(name="zb_sb", bufs=3) as zb_pool,
                tc.tile_pool(name="z_ps", bufs=2, space="PSUM") as z_ps,
                tc.tile_pool(name="h_ps", bufs=2, space="PSUM") as h_ps,
            ):
                for s in range(NSLAB):                # band = 16 dst nodes
                    xe = xe_pool.tile([128, SLAB], bf16, tag="xe")
                    if s == 0:
                        # split the first slab so compute starts after 512 KB
                        nc.sync.dma_start(
                            out=xe[:, 0:2048], in_=xe_d[:, 0:2048])
                        nc.sync.dma_start(
                            out=xe[:, 2048:SLAB], in_=xe_d[:, 2048:SLAB])
                        nc.sync.dma_start(out=xt[:], in_=xt_d[:])
                        nc.sync.dma_start(out=wself[:], in_=wself_d[:])
                        nc.sync.dma_start(out=wneigh[:], in_=wneigh_d[:])
                        nc.sync.dma_start(out=bsage[:], in_=bsage_d[:])
                    else:
                        nc.sync.dma_start(
                            out=xe[:], in_=xe_d[:, s * SLAB:(s + 1) * SLAB])
                    # A+B: 8 tiles of 1024 edge cols -> aggT[:, s*512:(s+1)*512]
                    for blk in range(SLAB // 1024):
                        ps = z_ps.tile([128, 1024], f32, tag="zps")
                        for hf in range(2):           # moving-dim cap is 512
                            nc.tensor.matmul(
                                out=ps[:, hf * 512:(hf + 1) * 512],
                                lhsT=wpool[:],
                                rhs=xe[:, blk * 1024 + hf * 512:
                                        blk * 1024 + (hf + 1) * 512],
                                start=True, stop=True,
                            )
                        t = s * (SLAB // 1024) + blk
                        if t % 2 == 0:
                            # scalar-engine path: psum -> bf16, fast DVE reduce
                            zb = zb_pool.tile([128, 1024], bf16, tag="zb")
                            nc.scalar.copy(out=zb[:], in_=ps[:])
                            nc.vector.reduce_max(
                                out=aggT[:, t * 64:(t + 1) * 64],
                                in_=zb[:].rearrange("p (n d) -> p n d", d=DEG),
                                axis=mybir.AxisListType.X)
                        else:
                            nc.vector.reduce_max(
                                out=aggT[:, t * 64:(t + 1) * 64],
                                in_=ps[:].rearrange("p (n d) -> p n d", d=DEG),
                                axis=mybir.AxisListType.X)
                    # B tail: fused bias + relu on this band (monotone max)
                    nc.scalar.activation(
                        out=aggT[:, s * 512:(s + 1) * 512],
                        in_=aggT[:, s * 512:(s + 1) * 512],
                        func=Relu, bias=bpool[:])
                    # D: ht for this band's 16 nodes (all 4 quadrants)
                    base = s * 512
                    for q in range(4):
                        psh = h_ps.tile([128, 512], f32, tag="hps")
                        nc.tensor.matmul(
                            out=psh[:],
                            lhsT=wself[:, q * H:(q + 1) * H],
                            rhs=xt[:, base: base + 512],
                            start=True, stop=False)
                        nc.tensor.matmul(
                            out=psh[:],
                            lhsT=wneigh[:, q * H:(q + 1) * H],
                            rhs=aggT[:, base: base + 512],
                            start=False, stop=True)
                        nc.scalar.activation(
                            out=htv[:, q, 16 * s:16 * s + 16, :],
                            in_=psh[:].rearrange("p (n gb) -> p n gb", gb=GB),
                            func=Relu, bias=bsage[:])
                    if s == 0:
                        # head weights: latest issue point before first E use
                        nc.sync.dma_start(out=wha[:], in_=wha_d[:])
                        nc.sync.dma_start(out=whv[:], in_=whv_d[:])
                    # E: accumulate head psums for this band's nodes
                    for j in range(16 * s, 16 * s + 16):
                        lhsT = ht[:, j * B:(j + 1) * B]
                        nc.tensor.matmul(
                            out=ps_adv[:],
                            lhsT=lhsT,
                            rhs=wha[:, j * NA:(j + 1) * NA],
                            start=(j == 0), stop=(j == NS - 1),
                        )
                        nc.tensor.matmul(
                            out=ps_val[:],
                            lhsT=lhsT,
                            rhs=whv[:, j * NV:(j + 1) * NV],
                            start=(j == 0), stop=(j == NS - 1),
                        )

            if _dbg:
                nc.sync.dma_start(out=dbg_aggT_d[:], in_=aggT[:])
            if _dbg:
                nc.sync.dma_start(out=dbg_ht_d[:], in_=ht[:])

            # ---- stage F: write partial heads (host sums across cores) ----
            with tc.tile_pool(name="tail", bufs=1) as tp:
                heads = tp.tile([128, NH], f32)
                nc.scalar.copy(out=heads[:, 0:NA], in_=ps_adv[:])
                nc.scalar.copy(out=heads[:, NA:NH], in_=ps_val[:])
                nc.sync.dma_start(out=heads_d[:], in_=heads[:])
            hd_ps_ctx.__exit__(None, None, None)
    nc.compile()
    return nc


def _make_in_maps(inputs):
    import ml_dtypes
    bf = ml_dtypes.bfloat16
    e4 = ml_dtypes.float8_e4m3

    x = np.asarray(inputs["x"], np.float32)
    src = np.asarray(inputs["src"], np.int32)
    W_pool = np.asarray(inputs["W_pool"], np.float32)
    b_pool = np.asarray(inputs["b_pool"], np.float32)
    W_self = np.asarray(inputs["W_self"], np.float32)
    W_neigh = np.asarray(inputs["W_neigh"], np.float32)
    b_sage = np.asarray(inputs["b_sage"], np.float32)
    W_adv = np.asarray(inputs["W_adv"], np.float32)
    b_adv = np.asarray(inputs["b_adv"], np.float32)
    W_v1 = np.asarray(inputs["W_v1"], np.float32)
    b_v1 = np.asarray(inputs["b_v1"], np.float32)
    W_v2 = np.asarray(inputs["W_v2"], np.float32)
    b_v2 = np.asarray(inputs["b_v2"], np.float32)
    W_v3 = np.asarray(inputs["W_v3"], np.float32)
    b_v3 = np.asarray(inputs["b_v3"], np.float32)

    # shared (replicated) tensors
    wpool_bd = np.kron(np.eye(4, dtype=np.float32), W_pool.T)                # [128, 128]
    wpool_bd = np.ascontiguousarray(wpool_bd).astype(bf)
    bpool = np.ascontiguousarray(np.tile(b_pool, 4)[:, None], np.float32)    # [128, 1]
    wself_bd = np.zeros((128, 4 * H), np.float32)                            # [128, 512]
    wneigh_bd = np.zeros((128, 4 * H), np.float32)
    for q in range(4):
        wself_bd[q * 32:(q + 1) * 32, q * H:(q + 1) * H] = W_self.T
        wneigh_bd[q * 32:(q + 1) * 32, q * H:(q + 1) * H] = W_neigh.T
    bsage = np.ascontiguousarray(b_sage[:, None])                            # [128, 1]
    shared = {
        "wpool_bd": wpool_bd, "bpool": bpool,
        "wself_bd": wself_bd.astype(bf), "wneigh_bd": wneigh_bd.astype(bf),
        "bsage": bsage,
    }

    # per-graph src indices (identical across graphs: dgl.batch of one graph)
    idxg = (src[: N * DEG] - 0).reshape(N, DEG)      # graph-0 global indices
    Wa = W_adv.reshape(NA, N, H)                     # [12, n, h]
    Wv = (W_v1 * VSCALE).reshape(NV, N, H)           # [64, n, h]

    in_maps = []
    for c in range(NCORES):
        sl = slice(c * NS, (c + 1) * NS)
        # xt[(q,f), n*GB+gb] = x[q*GB+gb, c*NS+n, f]
        xt = np.ascontiguousarray(
            x[:, sl, :].reshape(4, GB, NS, F).transpose(0, 3, 2, 1)
            .reshape(128, NS * GB)).astype(bf)
        # xe[(q,f), (n,gb,d)] = x[q*GB+gb, idxg[c*NS+n, d], f]
        gath = x[:, idxg[sl], :]                     # [128g, NS, DEG, F]
        xe = np.ascontiguousarray(
            gath.reshape(4, GB, NS, DEG, F).transpose(0, 4, 2, 1, 3)
            .reshape(128, NS * GB * DEG)).astype(bf)
        wha = np.ascontiguousarray(
            Wa[:, sl, :].transpose(2, 1, 0).reshape(H, NS * NA)).astype(bf)
        whv = np.ascontiguousarray(
            Wv[:, sl, :].transpose(2, 1, 0).reshape(H, NS * NV)).astype(e4)
        in_maps.append({"xt": xt, "xe": xe, "whead_adv": wha,
                        "whead_val": whv, **shared})
    return in_maps


def kernel(**inputs) -> np.ndarray:
    global LAST_RESULTS
    from concourse.bass_utils import run_bass_kernel_spmd

    if "nc" not in _CACHE:
        _CACHE["nc"] = _build_program()
    nc = _CACHE["nc"]
    in_maps = _make_in_maps(inputs)
    rr = run_bass_kernel_spmd(nc, in_maps, list(range(NCORES)))
    LAST_RESULTS = rr
    # unshard the K-split: sum the 8 partial head projections, then the
    # tiny dueling tail (~1 MFLOP) on the gathered result
    heads = np.zeros((B, NH), np.float32)
    for c in range(NCORES):
        heads += np.asarray(rr.results[c]["heads"], np.float32)
    b_adv = np.asarray(inputs["b_adv"], np.float32)
    b_v1 = np.asarray(inputs["b_v1"], np.float32)
    W_v2 = np.asarray(inputs["W_v2"], np.float32)
    b_v2 = np.asarray(inputs["b_v2"], np.float32)
    W_v3 = np.asarray(inputs["W_v3"], np.float32)
    b_v3 = np.asarray(inputs["b_v3"], np.float32)
    adv = np.maximum(heads[:, :NA] + b_adv, 0.0).reshape(B, 3, 4)
    val = np.maximum(heads[:, NA:] * (1.0 / VSCALE) + b_v1, 0.0)
    val = np.maximum(val @ W_v2.T + b_v2, 0.0)
    val = val @ W_v3.T + b_v3
    out = val[..., None] + adv - adv.mean(-1, keepdims=True)
    return np.ascontiguousarray(out, np.float32)
